# revision 5
# baseline (speedup 1.0000x reference)
"""Single-level 2D Haar DWT (periodization mode) on Trainium2 — bf16,
single-matmul butterfly.

Input x: (8, 512, 512, 16) fp32 NHWC. Output: (LL, LH, HL, HH), each
(8, 256, 256, 16) fp32 — +/- combinations of each 2x2 spatial block,
scaled by 0.5.

Sharding: pure data parallel — one batch sample per NeuronCore (8 cores).

The correctness gate is rel_err < 2e-2 and the kernel is HBM-bound, so
all device traffic runs in bf16 (halves both directions of HBM traffic
vs fp32; bf16 rounding contributes ~2e-3). The host does a conversion
pass over the input anyway, so it also reorders the layout for free:

  x[i] (512, 512, 16) -> (512 rows, 256 w-pairs, 2, 16) -> half-rows
  j = 2*row + w_parity, each 4096 elements. Quad j = 4q..4q+3 holds
  exactly the (a, b, c, d) contributors of output-row-quad q.

With the 2x2 block spread across four consecutive *partitions*, the
entire Haar butterfly (both spatial directions) is ONE matmul with a
fixed 128x128 +/-0.5 bf16 weight (the 0.5 scale folded in exactly):
PSUM partition s*32+q = subband s of quad q. Each output partition
line is one full contiguous output row of one subband, so output DMAs
are large and fully contiguous.

Per-core structure: 8 units of [128 half-rows x 4096 cols] (1 MB in,
1 MB out). Per unit: 1 input DMA -> 8 matmuls (PSUM, 2 groups of 4
banks) -> PSUM->SBUF bf16 copies alternating between ScalarE (ACT) and
VectorE (DVE) so neither engine exceeds ~20 us -> 4 output DMAs (one
per subband). Input DMAs ride the GpSimd SWDGE ring; output DMAs split
across the SP and ACT HWDGE rings. Tensor engine ~45 us busy, DMA
array ~52 us busy/engine — DMA-bound at the bf16 roofline.
"""

import sys

if "/opt/trn_rl_repo" not in sys.path:
    sys.path.insert(0, "/opt/trn_rl_repo")

import numpy as np
import ml_dtypes

BF16 = ml_dtypes.bfloat16

B, H, W, C = 8, 512, 512, 16
N_CORES = 8
HO, WO = H // 2, W // 2  # 256, 256
OROW = WO * C  # 4096 elements per output row
NJ = H * 2  # 1024 half-rows per sample, 4096 elements each

N_UNITS = 8
JPU = NJ // N_UNITS  # 128 half-rows (partitions) per unit
QPU = JPU // 4  # 32 quads (output rows) per unit

_CACHE = {}


def _haar_weight():
    """lhsT [k, m]: matmul computes out[m, n] = sum_k w[k, m] x[k, n].

    k = 4q+t with t = (a, b, c, d) of quad q; m = s*32 + q with
    s = (LL, LH, HL, HH). Signs per reference:
      LL = .5(a+b+c+d), LH = .5(a-b+c-d), HL = .5(a+b-c-d),
      HH = .5(a-b-c+d).
    """
    signs = {
        0: (1, 1, 1, 1),
        1: (1, -1, 1, -1),
        2: (1, 1, -1, -1),
        3: (1, -1, -1, 1),
    }
    w = np.zeros((128, 128), dtype=np.float32)
    for q in range(QPU):
        for s, sg in signs.items():
            for t in range(4):
                w[4 * q + t, s * QPU + q] = 0.5 * sg[t]
    return w.astype(BF16)


def _build():
    import concourse.bacc as bacc
    import concourse.mybir as mybir
    import concourse.tile as tile

    bf16 = mybir.dt.bfloat16
    fp32 = mybir.dt.float32

    nc = bacc.Bacc(
        "TRN2", target_bir_lowering=False, debug=False, num_devices=N_CORES
    )
    x = nc.dram_tensor("x", (NJ, OROW), bf16, kind="ExternalInput")
    wdram = nc.dram_tensor("w", (128, 128), bf16, kind="ExternalInput")
    outs = {
        name: nc.dram_tensor(name, (HO, OROW), bf16, kind="ExternalOutput")
        for name in ("LL", "LH", "HL", "HH")
    }

    GN = 2048  # PSUM group (4 banks fp32)
    MM_N = 512  # one fp32 matmul / PSUM bank
    BAND = 2 * OROW  # band tile = 2 units of 4096 cols
    SUBBANDS = ("LL", "LH", "HL", "HH")

    def emit_band(nc, pools, wt, b):
        """Band b covers units 2b and 2b+1 (256 half-rows of DRAM)."""
        inp, psum, outp = pools
        xt = inp.tile([128, BAND], bf16)
        ot = outp.tile([128, BAND], bf16)
        for u in range(2):
            k = 2 * b + u
            # chunked input: first matmuls start after 1 MB, not 2 MB
            nc.gpsimd.dma_start(
                xt[:, u * OROW : (u + 1) * OROW],
                x[k * JPU : (k + 1) * JPU, :],
            )
        for g in range(BAND // GN):  # 4 PSUM groups
            ps = psum.tile([128, GN], fp32)
            for j in range(GN // MM_N):
                lo = j * MM_N
                nc.tensor.matmul(
                    ps[:, lo : lo + MM_N],
                    wt[:],
                    xt[:, g * GN + lo : g * GN + lo + MM_N],
                    start=True,
                    stop=True,
                )
            dst = ot[:, g * GN : (g + 1) * GN]
            # alternate PSUM->SBUF bf16 evacuation between ACT and DVE
            if g % 2 == 0:
                nc.scalar.copy(dst, ps[:])
            else:
                nc.vector.tensor_copy(dst, ps[:])
        # one DMA per subband per band: [32 partitions, 2 chunks x 4096]
        rows = outs["LL"].shape[0] // 4  # 64 output rows per band
        for si, name in enumerate(SUBBANDS):
            src = ot[si * QPU : (si + 1) * QPU, :].rearrange(
                "p (u n) -> p u n", u=2
            )
            dst = (
                outs[name][b * rows : (b + 1) * rows, :]
                .rearrange("(u q) n -> q u n", u=2)
            )
            eng = nc.sync if si < 2 else nc.scalar
            eng.dma_start(dst, src)

    with tile.TileContext(nc) as tc:
        with (
            tc.tile_pool(name="wpool", bufs=1) as wpool,
            tc.tile_pool(name="inp", bufs=3) as inp,
            tc.tile_pool(name="psum", bufs=2, space="PSUM") as psum,
            tc.tile_pool(name="outp", bufs=2) as outp,
        ):
            wt = wpool.tile([128, 128], bf16)
            nc.gpsimd.dma_start(wt[:], wdram[:])
            pools = (inp, psum, outp)
            for b in range(N_UNITS // 2):
                emit_band(nc, pools, wt, b)

    nc.compile()
    return nc


def _get_nc():
    if "nc" not in _CACHE:
        _CACHE["nc"] = _build()
    return _CACHE["nc"]


def _in_maps(x):
    w = _haar_weight()
    # (B, H, W/2, 2, C) -> (B, H, 2, W/2, C): half-row j = 2*row + parity
    xb = (
        x.reshape(B, H, WO, 2, C)
        .transpose(0, 1, 3, 2, 4)
        .astype(BF16)
    )
    return [
        {"x": np.ascontiguousarray(xb[i].reshape(NJ, OROW)), "w": w}
        for i in range(B)
    ]


def kernel(x):
    from concourse.bass_utils import run_bass_kernel_spmd

    x = np.asarray(x, dtype=np.float32)
    assert x.shape == (B, H, W, C), x.shape

    nc = _get_nc()
    try:
        res = run_bass_kernel_spmd(nc, _in_maps(x), list(range(N_CORES)))
    except Exception:
        # transient NRT device errors have been observed right after
        # compile; one retry has always succeeded
        res = run_bass_kernel_spmd(nc, _in_maps(x), list(range(N_CORES)))

    out = []
    for name in ("LL", "LH", "HL", "HH"):
        out.append(
            np.stack(
                [
                    res.results[i][name]
                    .astype(np.float32)
                    .reshape(HO, WO, C)
                    for i in range(B)
                ],
                axis=0,
            )
        )
    return tuple(out)


# revision 6
# speedup vs baseline: 1.0778x; 1.0778x over previous
"""Single-level 2D Haar DWT (periodization mode) on Trainium2 — bf16,
hybrid TensorE/VectorE butterfly.

Input x: (8, 512, 512, 16) fp32 NHWC. Output: (LL, LH, HL, HH), each
(8, 256, 256, 16) fp32 — +/- combinations of each 2x2 spatial block,
scaled by 0.5.

Sharding: pure data parallel — one batch sample per NeuronCore (8 cores).

All device traffic runs in bf16 (gate is 2e-2; bf16 adds ~5e-3). The
host conversion pass also reorders layout for free. Work is split so no
single engine paces the pipeline (pure-matmul was Tensor-paced at
~41 us; DMA roofline is ~42 us/engine-busy):

M path (spatial rows 0..255) — TensorE:
  Half-rows j = (row, w-parity); quad j=4q..4q+3 holds the (a,b,c,d)
  of one 2x2 block, so ONE 128x128 +/-0.5 matmul computes all four
  subbands (scale folded exactly). PSUM partition s*32+t = subband s,
  quad t. ACT evacuates PSUM fp32 -> SBUF bf16. Host orders quads in
  parity-paired blocks (band = even-row block + odd-row block) so each
  output-DMA partition line holds TWO consecutive subband rows =
  16 KB contiguous DRAM writes.

D path (spatial rows 256..511) — VectorE:
  Host pre-scales by 0.5 (exact) and splits row pairs into top/bot
  line groups (16 KB DMA lines). Classic 8-op elementwise butterfly
  in bf16 (DVE 2x mode), no PSUM involved.

Per-core engine busy: Tensor ~21 us, DVE ~20 us, ACT ~18 us, DMA array
~42 us/engine — DMA-bound at the bf16 HBM roofline. Input DMAs +
D-path output DMAs ride the GpSimd SWDGE ring; M-path outputs the SP
HWDGE ring. Each subband gets its own DRAM tensor (writes to one
tensor serialize).
"""

import sys

if "/opt/trn_rl_repo" not in sys.path:
    sys.path.insert(0, "/opt/trn_rl_repo")

import numpy as np
import ml_dtypes

BF16 = ml_dtypes.bfloat16

B, H, W, C = 8, 512, 512, 16
N_CORES = 8
HO, WO = H // 2, W // 2  # 256, 256
OROW = WO * C  # 4096 elements per output row
ROW = W * C  # 8192 elements per input row

QPT = 32  # quads (output rows) per j-block

_CACHE = {}


def _haar_weight():
    """lhsT [k, m]: matmul computes out[m, n] = sum_k w[k, m] x[k, n].

    k = 4t+c with c = (a, b, c, d) of quad t; m = s*32 + t with
    s = (LL, LH, HL, HH). Signs per reference:
      LL = .5(a+b+c+d), LH = .5(a-b+c-d), HL = .5(a+b-c-d),
      HH = .5(a-b-c+d).
    """
    signs = {
        0: (1, 1, 1, 1),
        1: (1, -1, 1, -1),
        2: (1, 1, -1, -1),
        3: (1, -1, -1, 1),
    }
    w = np.zeros((128, 128), dtype=np.float32)
    for t in range(QPT):
        for s, sg in signs.items():
            for c in range(4):
                w[4 * t + c, s * QPT + t] = 0.5 * sg[c]
    return w.astype(BF16)


def _build():
    import concourse.bacc as bacc
    import concourse.mybir as mybir
    import concourse.tile as tile

    bf16 = mybir.dt.bfloat16
    fp32 = mybir.dt.float32

    nc = bacc.Bacc(
        "TRN2", target_bir_lowering=False, debug=False, num_devices=N_CORES
    )
    x_mm = nc.dram_tensor("x_mm", (512, OROW), bf16, kind="ExternalInput")
    x_dve = nc.dram_tensor("x_dve", (256, ROW), bf16, kind="ExternalInput")
    wdram = nc.dram_tensor("w", (128, 128), bf16, kind="ExternalInput")
    outs = {
        name: nc.dram_tensor(name, (HO, OROW), bf16, kind="ExternalOutput")
        for name in ("LL", "LH", "HL", "HH")
    }

    GN = 2048  # PSUM group (4 banks fp32)
    MM_N = 512  # one fp32 matmul / PSUM bank
    SUBBANDS = ("LL", "LH", "HL", "HH")

    def emit_m_band(nc, pools, wt, b):
        """Band b: j-blocks 2b (even out rows) + 2b+1 (odd), out rows
        64b..64b+64 of each subband."""
        minp, psum, motp = pools
        ot = motp.tile([128, 2 * OROW], bf16)
        for e in range(2):
            blk = 2 * b + e
            xt = minp.tile([128, OROW], bf16, tag=f"m{e}")
            nc.gpsimd.dma_start(
                xt[:], x_mm[blk * 128 : (blk + 1) * 128, :]
            )
            for g in range(OROW // GN):
                ps = psum.tile([128, GN], fp32)
                for j in range(GN // MM_N):
                    lo = j * MM_N
                    nc.tensor.matmul(
                        ps[:, lo : lo + MM_N],
                        wt[:],
                        xt[:, g * GN + lo : g * GN + lo + MM_N],
                        start=True,
                        stop=True,
                    )
                # ACT: PSUM fp32 -> SBUF bf16
                dst = ot[:, e * OROW + g * GN : e * OROW + (g + 1) * GN]
                nc.scalar.copy(dst, ps[:])
        rows = slice(64 * b, 64 * (b + 1))
        for si, name in enumerate(SUBBANDS):
            src = ot[si * QPT : (si + 1) * QPT, :].rearrange(
                "p (e n) -> p e n", e=2
            )
            dst = outs[name][rows, :].rearrange("(t e) n -> t e n", e=2)
            nc.sync.dma_start(dst, src)

    def emit_d_half(nc, tiles, h):
        """DVE butterfly on input cols h*4096..(h+1)*4096 (w-pairs
        h*128..+128) of spatial rows 256..511."""
        dt, db, dmid, otd = tiles
        cs = slice(h * OROW, (h + 1) * OROW)
        tv = dt[:, cs].rearrange("p (w u c) -> p w u c", u=2, c=C)
        bv = db[:, cs].rearrange("p (w u c) -> p w u c", u=2, c=C)
        a, bb = tv[:, :, 0, :], tv[:, :, 1, :]
        cc, dd = bv[:, :, 0, :], bv[:, :, 1, :]
        WH = OROW // (2 * C)  # 128 w-pairs per half
        t1 = dmid.tile([128, WH, C], bf16, tag="t1")
        t2 = dmid.tile([128, WH, C], bf16, tag="t2")
        u1 = dmid.tile([128, WH, C], bf16, tag="u1")
        u2 = dmid.tile([128, WH, C], bf16, tag="u2")
        nc.vector.tensor_add(t1[:], a, bb)
        nc.vector.tensor_add(t2[:], cc, dd)
        nc.vector.tensor_sub(u1[:], a, bb)
        nc.vector.tensor_sub(u2[:], cc, dd)
        oc = lambda s: slice(s * OROW + h * (OROW // 2), s * OROW + (h + 1) * (OROW // 2))
        for si, (i0, i1, op) in enumerate(
            ((t1, t2, "add"), (u1, u2, "add"), (t1, t2, "sub"), (u1, u2, "sub"))
        ):
            dst = otd[:, oc(si)].rearrange("p (w c) -> p w c", c=C)
            if op == "add":
                nc.vector.tensor_add(dst, i0[:], i1[:])
            else:
                nc.vector.tensor_sub(dst, i0[:], i1[:])

    with tile.TileContext(nc) as tc:
        with (
            tc.tile_pool(name="wpool", bufs=1) as wpool,
            tc.tile_pool(name="dput", bufs=1) as dput,
            tc.tile_pool(name="minp", bufs=3) as minp,
            tc.tile_pool(name="psum", bufs=2, space="PSUM") as psum,
            tc.tile_pool(name="motp", bufs=2) as motp,
            tc.tile_pool(name="dmid", bufs=2) as dmid,
        ):
            wt = wpool.tile([128, 128], bf16)
            nc.gpsimd.dma_start(wt[:], wdram[:])
            dt = dput.tile([128, ROW], bf16, tag="dt")
            db = dput.tile([128, ROW], bf16, tag="db")
            otd = dput.tile([128, 4 * OROW], bf16, tag="otd")
            # D-path first col-half input
            nc.gpsimd.dma_start(dt[:, 0:OROW], x_dve[0:128, 0:OROW])
            nc.gpsimd.dma_start(db[:, 0:OROW], x_dve[128:256, 0:OROW])
            m_pools = (minp, psum, motp)
            d_tiles = (dt, db, dmid, otd)
            emit_m_band(nc, m_pools, wt, 0)
            # D-path second col-half input + first-half butterfly
            nc.gpsimd.dma_start(dt[:, OROW:ROW], x_dve[0:128, OROW:ROW])
            nc.gpsimd.dma_start(db[:, OROW:ROW], x_dve[128:256, OROW:ROW])
            emit_d_half(nc, d_tiles, 0)
            emit_m_band(nc, m_pools, wt, 1)
            emit_d_half(nc, d_tiles, 1)
            # D-path outputs: subband rows 128..255, full rows
            for si, name in enumerate(SUBBANDS):
                nc.gpsimd.dma_start(
                    outs[name][128:256, :],
                    otd[:, si * OROW : (si + 1) * OROW],
                )

    nc.compile()
    return nc


def _get_nc():
    if "nc" not in _CACHE:
        _CACHE["nc"] = _build()
    return _CACHE["nc"]


def _in_maps(x):
    w = _haar_weight()
    maps = []
    for i in range(B):
        xs = x[i]  # (512, 512, 16) fp32
        # M path: rows 0..256 -> j-layout with parity-paired blocks.
        # axes: (b, t, e, rp, wp, u, c); j order = (b, e, t, rp, u)
        xm = (
            xs[:256]
            .reshape(2, 32, 2, 2, WO, 2, C)
            .transpose(0, 2, 1, 3, 5, 4, 6)
            .reshape(512, OROW)
            .astype(BF16)
        )
        # D path: rows 256..512, pre-scaled by 0.5, top lines then bot
        xd = (
            (xs[256:] * np.float32(0.5))
            .reshape(128, 2, ROW)
            .transpose(1, 0, 2)
            .reshape(256, ROW)
            .astype(BF16)
        )
        maps.append(
            {
                "x_mm": np.ascontiguousarray(xm),
                "x_dve": np.ascontiguousarray(xd),
                "w": w,
            }
        )
    return maps


def kernel(x):
    from concourse.bass_utils import run_bass_kernel_spmd

    x = np.asarray(x, dtype=np.float32)
    assert x.shape == (B, H, W, C), x.shape

    nc = _get_nc()
    try:
        res = run_bass_kernel_spmd(nc, _in_maps(x), list(range(N_CORES)))
    except Exception:
        # transient NRT device errors have been observed right after
        # compile; one retry has always succeeded
        res = run_bass_kernel_spmd(nc, _in_maps(x), list(range(N_CORES)))

    out = []
    for name in ("LL", "LH", "HL", "HH"):
        out.append(
            np.stack(
                [
                    res.results[i][name]
                    .astype(np.float32)
                    .reshape(HO, WO, C)
                    for i in range(B)
                ],
                axis=0,
            )
        )
    return tuple(out)


# revision 11
# speedup vs baseline: 1.1058x; 1.0260x over previous
"""Single-level 2D Haar DWT (periodization mode) on Trainium2 — bf16,
hybrid TensorE/VectorE butterfly.

Input x: (8, 512, 512, 16) fp32 NHWC. Output: (LL, LH, HL, HH), each
(8, 256, 256, 16) fp32 — +/- combinations of each 2x2 spatial block,
scaled by 0.5.

Sharding: pure data parallel — one batch sample per NeuronCore (8 cores).

All device traffic runs in bf16 (gate is 2e-2; bf16 adds ~5e-3). The
host conversion pass also reorders layout for free. Work is split so no
single engine paces the pipeline (pure-matmul was Tensor-paced at
~41 us; DMA roofline is ~42 us/engine-busy):

M path (spatial rows 0..255) — TensorE:
  Half-rows j = (row, w-parity); quad j=4q..4q+3 holds the (a,b,c,d)
  of one 2x2 block, so ONE 128x128 +/-0.5 matmul computes all four
  subbands (scale folded exactly). PSUM partition s*32+t = subband s,
  quad t. ACT evacuates PSUM fp32 -> SBUF bf16. Host orders quads in
  parity-paired blocks (band = even-row block + odd-row block) so each
  output-DMA partition line holds TWO consecutive subband rows =
  16 KB contiguous DRAM writes.

D path (spatial rows 256..511) — VectorE:
  Host pre-scales by 0.5 (exact) and splits row pairs into top/bot
  line groups (16 KB DMA lines). Classic 8-op elementwise butterfly
  in bf16 (DVE 2x mode), no PSUM involved.

Per-core engine busy: Tensor ~21 us, DVE ~20 us, ACT ~18 us, DMA array
~42 us/engine — DMA-bound at the bf16 HBM roofline. Input DMAs +
D-path output DMAs ride the GpSimd SWDGE ring; M-path outputs the SP
HWDGE ring. Each subband gets its own DRAM tensor (writes to one
tensor serialize).
"""

import sys

if "/opt/trn_rl_repo" not in sys.path:
    sys.path.insert(0, "/opt/trn_rl_repo")

import numpy as np
import ml_dtypes

BF16 = ml_dtypes.bfloat16

B, H, W, C = 8, 512, 512, 16
N_CORES = 8
HO, WO = H // 2, W // 2  # 256, 256
OROW = WO * C  # 4096 elements per output row
ROW = W * C  # 8192 elements per input row

QPT = 32  # quads (output rows) per j-block

_CACHE = {}


def _haar_weight():
    """lhsT [k, m]: matmul computes out[m, n] = sum_k w[k, m] x[k, n].

    k = 4t+c with c = (a, b, c, d) of quad t; m = s*32 + t with
    s = (LL, LH, HL, HH). Signs per reference:
      LL = .5(a+b+c+d), LH = .5(a-b+c-d), HL = .5(a+b-c-d),
      HH = .5(a-b-c+d).
    """
    signs = {
        0: (1, 1, 1, 1),
        1: (1, -1, 1, -1),
        2: (1, 1, -1, -1),
        3: (1, -1, -1, 1),
    }
    w = np.zeros((128, 128), dtype=np.float32)
    for t in range(QPT):
        for s, sg in signs.items():
            for c in range(4):
                w[4 * t + c, s * QPT + t] = 0.5 * sg[c]
    return w.astype(BF16)


def _build():
    import concourse.bacc as bacc
    import concourse.mybir as mybir
    import concourse.tile as tile

    bf16 = mybir.dt.bfloat16
    fp32 = mybir.dt.float32

    nc = bacc.Bacc(
        "TRN2", target_bir_lowering=False, debug=False, num_devices=N_CORES
    )
    x_mm = nc.dram_tensor("x_mm", (512, OROW), bf16, kind="ExternalInput")
    x_dve = nc.dram_tensor("x_dve", (256, ROW), bf16, kind="ExternalInput")
    wdram = nc.dram_tensor("w", (128, 128), bf16, kind="ExternalInput")
    outs = {
        name: nc.dram_tensor(name, (HO, OROW), bf16, kind="ExternalOutput")
        for name in ("LL", "LH", "HL", "HH")
    }

    GN = 2048  # PSUM group (4 banks fp32)
    MM_N = 512  # one fp32 matmul / PSUM bank
    SUBBANDS = ("LL", "LH", "HL", "HH")

    def emit_m_block(nc, pools, wt, blk, xt):
        """j-block blk -> out rows 32*blk..+32 of each subband."""
        minp, psum, motp = pools
        ot = motp.tile([128, OROW], bf16)
        for g in range(OROW // GN):
            ps = psum.tile([128, GN], fp32)
            for j in range(GN // MM_N):
                lo = j * MM_N
                nc.tensor.matmul(
                    ps[:, lo : lo + MM_N],
                    wt[:],
                    xt[:, g * GN + lo : g * GN + lo + MM_N],
                    start=True,
                    stop=True,
                )
            # ACT: PSUM fp32 -> SBUF bf16
            nc.scalar.copy(ot[:, g * GN : (g + 1) * GN], ps[:])
        rows = slice(32 * blk, 32 * (blk + 1))
        for si, name in enumerate(SUBBANDS):
            nc.sync.dma_start(
                outs[name][rows, :], ot[si * QPT : (si + 1) * QPT, :]
            )

    def emit_d_half(nc, tiles, h):
        """DVE butterfly on input cols h*4096..(h+1)*4096 (w-pairs
        h*128..+128) of spatial rows 256..511."""
        dt, db, dmid, otd = tiles
        cs = slice(h * OROW, (h + 1) * OROW)
        tv = dt[:, cs].rearrange("p (w u c) -> p w u c", u=2, c=C)
        bv = db[:, cs].rearrange("p (w u c) -> p w u c", u=2, c=C)
        a, bb = tv[:, :, 0, :], tv[:, :, 1, :]
        cc, dd = bv[:, :, 0, :], bv[:, :, 1, :]
        WH = OROW // (2 * C)  # 128 w-pairs per half
        t1 = dmid.tile([128, WH, C], bf16, tag="t1")
        t2 = dmid.tile([128, WH, C], bf16, tag="t2")
        u1 = dmid.tile([128, WH, C], bf16, tag="u1")
        u2 = dmid.tile([128, WH, C], bf16, tag="u2")
        nc.vector.tensor_add(t1[:], a, bb)
        nc.vector.tensor_add(t2[:], cc, dd)
        nc.vector.tensor_sub(u1[:], a, bb)
        nc.vector.tensor_sub(u2[:], cc, dd)
        oc = lambda s: slice(s * OROW + h * (OROW // 2), s * OROW + (h + 1) * (OROW // 2))
        for si, (i0, i1, op) in enumerate(
            ((t1, t2, "add"), (u1, u2, "add"), (t1, t2, "sub"), (u1, u2, "sub"))
        ):
            dst = otd[:, oc(si)].rearrange("p (w c) -> p w c", c=C)
            if op == "add":
                nc.vector.tensor_add(dst, i0[:], i1[:])
            else:
                nc.vector.tensor_sub(dst, i0[:], i1[:])

    with tile.TileContext(nc) as tc:
        with (
            tc.tile_pool(name="wpool", bufs=1) as wpool,
            tc.tile_pool(name="dput", bufs=1) as dput,
            tc.tile_pool(name="minp", bufs=4) as minp,
            tc.tile_pool(name="psum", bufs=2, space="PSUM") as psum,
            tc.tile_pool(name="motp", bufs=2) as motp,
            tc.tile_pool(name="dmid", bufs=2) as dmid,
        ):
            wt = wpool.tile([128, 128], bf16)
            nc.gpsimd.dma_start(wt[:], wdram[:])
            dt = dput.tile([128, ROW], bf16, tag="dt")
            db = dput.tile([128, ROW], bf16, tag="db")
            otd = dput.tile([128, 4 * OROW], bf16, tag="otd")
            m_pools = (minp, psum, motp)
            d_tiles = (dt, db, dmid, otd)

            # interleave M-block and D-half inputs on the gpsimd queue so
            # the matmul path starts right away and the D butterfly has
            # its first col-half early
            mt = []
            def m_in(blk):
                t = minp.tile([128, OROW], bf16)
                nc.gpsimd.dma_start(t[:], x_mm[blk * 128 : (blk + 1) * 128, :])
                mt.append(t)

            nc.gpsimd.dma_start(dt[:, 0:OROW], x_dve[0:128, 0:OROW])
            m_in(0)
            nc.gpsimd.dma_start(db[:, 0:OROW], x_dve[128:256, 0:OROW])
            m_in(1)
            nc.gpsimd.dma_start(dt[:, OROW:ROW], x_dve[0:128, OROW:ROW])
            m_in(2)
            nc.gpsimd.dma_start(db[:, OROW:ROW], x_dve[128:256, OROW:ROW])
            m_in(3)

            def d_outs(h):
                # half h -> out cols h*2048..+2048 of subband rows 128..255
                lo, hi = h * (OROW // 2), (h + 1) * (OROW // 2)
                for si, name in enumerate(SUBBANDS):
                    nc.sync.dma_start(
                        outs[name][128:256, lo:hi],
                        otd[:, si * OROW + lo : si * OROW + hi],
                    )

            emit_m_block(nc, m_pools, wt, 0, mt[0])
            emit_d_half(nc, d_tiles, 0)
            d_outs(0)
            emit_m_block(nc, m_pools, wt, 1, mt[1])
            emit_m_block(nc, m_pools, wt, 2, mt[2])
            emit_d_half(nc, d_tiles, 1)
            d_outs(1)
            emit_m_block(nc, m_pools, wt, 3, mt[3])

    nc.compile()
    return nc


def _get_nc():
    if "nc" not in _CACHE:
        _CACHE["nc"] = _build()
    return _CACHE["nc"]


def _in_maps(x):
    w = _haar_weight()
    maps = []
    for i in range(B):
        xs = x[i]  # (512, 512, 16) fp32
        # M path: rows 0..256 -> j-layout, block blk = out rows 32blk..+32.
        # axes: (blk, t, rp, wp, u, c); j order = (blk, t, rp, u)
        xm = (
            xs[:256]
            .reshape(4, 32, 2, WO, 2, C)
            .transpose(0, 1, 2, 4, 3, 5)
            .reshape(512, OROW)
            .astype(BF16)
        )
        # D path: rows 256..512, pre-scaled by 0.5, top lines then bot
        xd = (
            (xs[256:] * np.float32(0.5))
            .reshape(128, 2, ROW)
            .transpose(1, 0, 2)
            .reshape(256, ROW)
            .astype(BF16)
        )
        maps.append(
            {
                "x_mm": np.ascontiguousarray(xm),
                "x_dve": np.ascontiguousarray(xd),
                "w": w,
            }
        )
    return maps


def kernel(x):
    from concourse.bass_utils import run_bass_kernel_spmd

    x = np.asarray(x, dtype=np.float32)
    assert x.shape == (B, H, W, C), x.shape

    nc = _get_nc()
    try:
        res = run_bass_kernel_spmd(nc, _in_maps(x), list(range(N_CORES)))
    except Exception:
        # transient NRT device errors have been observed right after
        # compile; one retry has always succeeded
        res = run_bass_kernel_spmd(nc, _in_maps(x), list(range(N_CORES)))

    out = []
    for name in ("LL", "LH", "HL", "HH"):
        out.append(
            np.stack(
                [
                    res.results[i][name]
                    .astype(np.float32)
                    .reshape(HO, WO, C)
                    for i in range(B)
                ],
                axis=0,
            )
        )
    return tuple(out)
